# revision 70
# baseline (speedup 1.0000x reference)
"""GNN message-passing kernel for 8 Trainium2 NeuronCores (v4).

Reference semantics:
    h = tanh(node_feat @ w_n2l + b_n2l)
    for lv in range(3):
        conv = (h @ conv_w[lv] + conv_b[lv]).reshape(N, 4, D)
        msgs[e] = segment_sum(conv[:, e, :][src_e], dst_e, N)
        msg = tanh(concat_e(msgs))
        h = tanh(msg @ merge_w[lv] + merge_b[lv] + h)

v4 design (h-gather, region streams, PSUM-resident merge):
  - Each level: AllGather the raw h shard in bf16 -> hfull [8*SP, 128]
    (25.7MB).  Conv is applied AFTER the per-type segment sum
    (linearity): msgs_e^T = conv_w_e^T @ seg_e^T + conv_b_e x deg_e.
  - Gather elements are whole h rows (256B), so one gather stream per
    src pair-region serves ALL 4 edge types; slots are ordered by
    (dst block w, edge type e).  4 streams ride 4 SWDGE queues.
  - Loop order w-outer / e-inner: the merge matmul accumulates all 4
    types (+ merge bias + residual via identity matmul) in one PSUM
    tile per block - no SBUF accumulator, no separate merge pass.
  - Finish chains are deferred one group so ACT/PE ping-pong overlaps
    the next group's scatter matmuls.
  - Pad slots use idx=0 / dstoff=-1 (zero one-hot column).

Distribution: nodes sharded over 8 cores (12500 each). Weights replicated.
"""

import math
import numpy as np
import ml_dtypes

BF16 = ml_dtypes.bfloat16

# ----------------------------------------------------------------------------
# configuration
# ----------------------------------------------------------------------------


class Cfg:
    def __init__(self, n_nodes=100000, n_cores=8, ept=1600000, piece_chunks=16,
                 nqueues=4, sort_src=True, single_packet=False, lookahead=2):
        self.N = n_nodes
        self.NC = n_cores
        self.ET = 4          # edge types
        self.LV = 3          # levels
        self.D = 128         # feature dim (latent == feats == 128)
        self.EPT = ept       # edges per type
        self.S = self.N // self.NC            # real nodes per core
        self.SP = ((self.S + 127) // 128) * 128   # padded nodes per core
        self.W = self.SP // 128               # dst blocks per core
        self.NREG = self.NC // 2              # src core-pair regions
        self.REG_ROWS = 2 * self.SP           # rows per pair region
        assert self.REG_ROWS - 1 <= 32767, "int16 gather index limit"
        self.PIECE_CH = piece_chunks          # chunks per gather piece
        self.PIECE = self.PIECE_CH * 128      # slots per gather piece
        self.NQ = nqueues                     # SWDGE queues for gathers
        self.SORT_SRC = sort_src              # sort slots by src within group
        self.SINGLE_PACKET = single_packet
        self.LA = lookahead                   # gather piece lookahead depth


# ----------------------------------------------------------------------------
# CPU preprocessing: edge bucketing, capacities, index/offset arrays
# ----------------------------------------------------------------------------


def preprocess_edges(cfg, edge_src, edge_dst):
    """Bucket edges by (dst core, src pair-region, dst block, edge type).

    One stream per src pair-region; slots ordered by (w, e, rank).
    Returns a plan shared by all cores (capacities, piece table) plus
    per-core idx / dstoff arrays.
    """
    NC, ET, W, S, SP = cfg.NC, cfg.ET, cfg.W, cfg.S, cfg.SP
    NREG = cfg.NREG

    m = np.zeros((NC, ET, NREG, W), dtype=np.int64)
    per_core_edges = [[None] * ET for _ in range(NC)]  # (scp, w, idx16, doff)
    for e in range(ET):
        src = np.asarray(edge_src[e], dtype=np.int64)
        dst = np.asarray(edge_dst[e], dtype=np.int64)
        owner = dst // S
        dl = dst - owner * S
        w = dl >> 7
        scp = src // (2 * S)
        src_in_pair = src - scp * (2 * S)
        idx16 = (src_in_pair // S) * SP + (src_in_pair % S)
        doff = dl & 127
        key = (owner * NREG + scp) * W + w
        cnt = np.bincount(key, minlength=NC * NREG * W).reshape(NC, NREG, W)
        m[:, e] = cnt
        order = np.argsort(key, kind="stable")
        ksorted = key[order]
        bounds = np.searchsorted(ksorted, np.arange(NC) * NREG * W)
        bounds = np.append(bounds, len(ksorted))
        for c in range(NC):
            sl = order[bounds[c]:bounds[c + 1]]
            per_core_edges[c][e] = (
                scp[sl], w[sl], idx16[sl].astype(np.int16),
                doff[sl].astype(np.int16)
            )

    # capacities (chunks of 128) per (e, scp, w): max over cores
    K = np.ceil(m.max(axis=0) / 128.0).astype(np.int64)  # [ET, NREG, W]

    # stream layout per scp: groups ordered by (w, e)
    group_slot_start = np.zeros((NREG, W, ET), dtype=np.int64)
    stream_len = np.zeros(NREG, dtype=np.int64)
    for scp in range(NREG):
        off = 0
        for w in range(W):
            for e in range(ET):
                group_slot_start[scp, w, e] = off
                off += K[e, scp, w] * 128
        stream_len[scp] = off

    # piece table: per stream, pieces of cfg.PIECE slots (last short)
    pieces = []  # (scp, slot_start, n_slots)
    piece_of_stream = {}
    for scp in range(NREG):
        start_list = []
        off = 0
        while off < stream_len[scp]:
            n = min(cfg.PIECE, stream_len[scp] - off)
            n = ((n + 127) // 128) * 128
            start_list.append((len(pieces), off, n))
            pieces.append((scp, off, n))
            off += n
        piece_of_stream[scp] = start_list

    npieces = len(pieces)
    idx_cols = cfg.PIECE // 16
    off_cols = cfg.PIECE_CH

    # per-core arrays
    idx_arrs = []
    off_arrs = []
    deg_arrs = []
    for c in range(NC):
        idx_flat = {scp: np.zeros(stream_len[scp], dtype=np.int16)
                    for scp in range(NREG)}
        off_flat = {scp: np.full(stream_len[scp], -1, dtype=np.int16)
                    for scp in range(NREG)}
        deg = np.zeros((cfg.ET, cfg.SP), dtype=np.float32)
        for e in range(ET):
            escp, ew, eidx, edoff = per_core_edges[c][e]
            gkey = escp * W + ew
            if cfg.SORT_SRC:
                order = np.lexsort((eidx, gkey))
            else:
                order = np.argsort(gkey, kind="stable")
            gs = gkey[order]
            grp_start_pos = np.searchsorted(gs, gs, side="left")
            rank = np.arange(len(gs)) - grp_start_pos
            slot = group_slot_start[escp[order], ew[order], e] + rank
            for scp in range(NREG):
                msk = escp[order] == scp
                idx_flat[scp][slot[msk]] = eidx[order][msk]
                off_flat[scp][slot[msk]] = edoff[order][msk]
            dln = ew * 128 + edoff
            deg[e] = np.bincount(dln, minlength=cfg.SP).astype(np.float32)
        # wrap into DMA layouts
        idx_arr = np.zeros((npieces, 128, idx_cols), dtype=np.int16)
        off_arr = np.full((npieces, 128, off_cols), -1, dtype=np.int16)
        for scp, plist in piece_of_stream.items():
            fl_i = idx_flat[scp]
            fl_o = off_flat[scp]
            for (pid, off0, n) in plist:
                chunk = np.zeros(n, dtype=np.int16)
                avail = min(n, len(fl_i) - off0)
                chunk[:avail] = fl_i[off0:off0 + avail]
                wrapped = chunk.reshape(-1, 16).T          # [16, n/16]
                idx_arr[pid, :, : n // 16] = np.tile(wrapped, (8, 1))
                oc = np.full(n, -1, dtype=np.int16)
                oc[:avail] = fl_o[off0:off0 + avail]
                oc = oc.reshape(-1, 128).T                # [128, n/128]
                off_arr[pid, :, : n // 128] = oc
        idx_arrs.append(idx_arr)
        off_arrs.append(off_arr)
        # deg packed per w: [W, ET*128]
        degw = np.ascontiguousarray(
            deg.reshape(ET, W, 128).transpose(1, 0, 2).reshape(W, ET * 128))
        deg_arrs.append(degw)

    plan = {
        "K": K,
        "group_slot_start": group_slot_start,
        "stream_len": stream_len,
        "pieces": pieces,
        "piece_of_stream": piece_of_stream,
        "npieces": npieces,
        "idx_cols": idx_cols,
        "off_cols": off_cols,
    }
    return plan, idx_arrs, off_arrs, deg_arrs


# ----------------------------------------------------------------------------
# program builder
# ----------------------------------------------------------------------------


def build_program(cfg, plan, ablate=()):
    ablate = set(ablate)
    from concourse import bass, bacc, tile, mybir

    f32 = mybir.dt.float32
    b16 = mybir.dt.bfloat16
    i16 = mybir.dt.int16
    Tanh = mybir.ActivationFunctionType.Tanh
    Copy = mybir.ActivationFunctionType.Copy

    NC, ET, W, LV = cfg.NC, cfg.ET, cfg.W, cfg.LV
    NREG = cfg.NREG
    SP = cfg.SP
    K = plan["K"]
    group_slot_start = plan["group_slot_start"]
    piece_of_stream = plan["piece_of_stream"]
    npieces = plan["npieces"]
    idx_cols = plan["idx_cols"]
    off_cols = plan["off_cols"]

    nc = bacc.Bacc("TRN2", target_bir_lowering=False, debug=False,
                   num_devices=NC, num_swdge_queues=cfg.NQ)

    # ---- external tensors ----
    nf_t = nc.dram_tensor("node_feat_t", [128, SP], f32, kind="ExternalInput")
    idx_d = nc.dram_tensor("idx", [npieces, 128, idx_cols], i16,
                           kind="ExternalInput")
    off_d = nc.dram_tensor("dstoff", [npieces, 128, off_cols], i16,
                           kind="ExternalInput")
    deg_d = nc.dram_tensor("deg", [W, ET * 128], f32, kind="ExternalInput")
    w_n2l_d = nc.dram_tensor("w_n2l", [128, 128], f32, kind="ExternalInput")
    b_n2l_d = nc.dram_tensor("b_n2l", [1, 128], f32, kind="ExternalInput")
    ident_d = nc.dram_tensor("ident", [128, 128], f32, kind="ExternalInput")
    convw_d = nc.dram_tensor("conv_w", [LV, 128, ET * 128], f32,
                             kind="ExternalInput")
    convb_d = nc.dram_tensor("conv_b", [LV, 1, ET * 128], f32,
                             kind="ExternalInput")
    mw_d = nc.dram_tensor("merge_w", [LV, 128, ET * 128], f32,
                          kind="ExternalInput")
    mb_d = nc.dram_tensor("merge_b", [LV, 1, 128], f32, kind="ExternalInput")
    out_d = nc.dram_tensor("out", [SP, 128], f32, kind="ExternalOutput")

    with tile.TileContext(nc) as tc:
        with (
            tc.tile_pool(name="dram2", bufs=2, space="DRAM") as dramp2,
            tc.tile_pool(name="const", bufs=1) as constp,
            tc.tile_pool(name="wts", bufs=2) as wtsp,
            tc.tile_pool(name="gather", bufs=cfg.LA + 1) as gatherp,
            tc.tile_pool(name="ponehot", bufs=cfg.LA + 1) as ponep,
            tc.tile_pool(name="idxp", bufs=cfg.LA + 1) as idxp,
            tc.tile_pool(name="small", bufs=3) as smallp,
            tc.tile_pool(name="psum_s", bufs=3, space="PSUM") as psum_s_pool,
            tc.tile_pool(name="psum_m", bufs=2, space="PSUM") as psum_m_pool,
            tc.tile_pool(name="psum_cv", bufs=3, space="PSUM") as psum_cv_pool,
        ):
            # ---- DRAM intermediates ----
            hb_cur = dramp2.tile([SP, 128], b16, tag="hb", name="hb_cur")
            h_cur = dramp2.tile([SP, 128], f32, tag="h_cur", name="h_cur")

            # ---- constants ----
            iota_t = constp.tile([128, 128], i16)
            nc.gpsimd.iota(iota_t[:], pattern=[[1, 128]], base=0,
                           channel_multiplier=0)
            ones_f = constp.tile([1, 128], f32)
            nc.vector.memset(ones_f[:], 1.0)
            ident_t = constp.tile([128, 128], f32)
            nc.sync.dma_start(ident_t[:], ident_d[:])
            w_n2l_t = constp.tile([128, 128], f32)
            nc.sync.dma_start(w_n2l_t[:], w_n2l_d[:])
            b_n2l_t = constp.tile([1, 128], f32)
            nc.sync.dma_start(b_n2l_t[:], b_n2l_d[:])

            # ---- embed: h0 = tanh(nf @ w_n2l + b_n2l) ----
            # nf tile is the stationary operand, so the result lands in
            # row layout [node, f] directly - no transpose pass.
            for w in range(W):
                nf_w = smallp.tile([128, 128], f32, tag="nf", name="nf_w")
                nc.sync.dma_start(nf_w[:], nf_t[:, w * 128:(w + 1) * 128])
                ps_h = psum_m_pool.tile([128, 128], f32, tag="ps_m",
                                        name="ps_h")
                nc.tensor.matmul(ps_h[:], nf_w[:], w_n2l_t[:],
                                 start=True, stop=False)
                nc.tensor.matmul(ps_h[:], ones_f[:], b_n2l_t[:],
                                 start=False, stop=True)
                h_w = smallp.tile([128, 128], f32, tag="hrow", name="h_w")
                nc.scalar.activation(h_w[:], ps_h[:], Tanh)
                nc.sync.dma_start(h_cur[w * 128:(w + 1) * 128, :], h_w[:])
                hb_w = smallp.tile([128, 128], b16, tag="hbrow", name="hb_w")
                nc.scalar.activation(hb_w[:], ps_h[:], Tanh)
                nc.sync.dma_start(hb_cur[w * 128:(w + 1) * 128, :], hb_w[:])

            # ---- level loop ----
            for lv in range(LV if "levels" not in ablate else 0):
                hfull = dramp2.tile([NC * SP, 128], b16, tag="hfull",
                                    name="hfull", addr_space="Shared")
                if "allgather" not in ablate:
                    nc.gpsimd.collective_compute(
                        "AllGather",
                        bass.mybir.AluOpType.bypass,
                        replica_groups=[list(range(NC))],
                        ins=[hb_cur[:].opt()],
                        outs=[hfull[:].opt()],
                    )

                convw_t = wtsp.tile([128, ET * 128], f32, tag="convw",
                                    name="convw_t")
                nc.sync.dma_start(convw_t[:], convw_d[lv])
                convb_t = wtsp.tile([1, ET * 128], f32, tag="convb",
                                    name="convb_t")
                nc.sync.dma_start(convb_t[:], convb_d[lv])
                mw_t = wtsp.tile([128, ET * 128], f32, tag="mw", name="mw_t")
                nc.sync.dma_start(mw_t[:], mw_d[lv])
                mb_t = wtsp.tile([1, 128], f32, tag="mb", name="mb_t")
                nc.sync.dma_start(mb_t[:], mb_d[lv])

                h_nxt = (dramp2.tile([SP, 128], f32, tag="h_cur",
                                     name="h_nxt")
                         if lv < LV - 1 else None)
                hb_nxt = (dramp2.tile([SP, 128], b16, tag="hb",
                                      name="hb_nxt")
                          if lv < LV - 1 else None)

                # ---- per-level gather stream state ----
                nring = cfg.LA + 1
                issued = [-1] * NREG
                ring_g = [[None] * nring for _ in range(NREG)]
                ring_p = [[None] * nring for _ in range(NREG)]

                def issue_piece(scp, pi):
                    plist = piece_of_stream[scp]
                    if pi >= len(plist):
                        issued[scp] = pi
                        return
                    gpid, off0, n = plist[pi]
                    nch = n // 128
                    it = idxp.tile([128, idx_cols], i16, tag=f"idx{scp}",
                                   name="it")
                    if "gather" not in ablate:
                        nc.sync.dma_start(it[:, : n // 16],
                                          idx_d[gpid][:, : n // 16])
                    ot = idxp.tile([128, off_cols], i16, tag=f"off{scp}",
                                   name="ot")
                    if "onehot" not in ablate:
                        nc.sync.dma_start(ot[:, :nch], off_d[gpid][:, :nch])
                    gt = gatherp.tile([128, cfg.PIECE_CH, 128], b16,
                                      tag=f"g{scp}", name="gt")
                    reg_base = scp * cfg.REG_ROWS
                    if "gather" in ablate:
                        nc.vector.memset(gt[:, :nch, 0:1], 0.0)
                    else:
                        nc.gpsimd.dma_gather(
                            out_ap=gt[:, :nch, :],
                            in_ap=hfull[reg_base:reg_base + cfg.REG_ROWS, :],
                            idxs_ap=it[:, : n // 16],
                            num_idxs=n,
                            num_idxs_reg=n,
                            elem_size=128,
                            elem_step=128,
                            single_packet=cfg.SINGLE_PACKET,
                            queue_num=scp % cfg.NQ,
                        )
                    pt = ponep.tile([128, cfg.PIECE_CH * 128], b16,
                                    tag=f"p{scp}", name="pt")
                    if "onehot" in ablate:
                        nc.vector.memset(pt[:, 0:1], 0.0)
                    else:
                        nc.vector.tensor_tensor(
                            out=pt[:, : nch * 128].rearrange(
                                "p (c e) -> p c e", e=128),
                            in0=ot[:, :nch].unsqueeze(2).to_broadcast(
                                [128, nch, 128]),
                            in1=iota_t[:].unsqueeze(1).to_broadcast(
                                [128, nch, 128]),
                            op=bass.mybir.AluOpType.is_equal,
                        )
                    issued[scp] = pi
                    ring_g[scp][pi % nring] = gt
                    ring_p[scp][pi % nring] = pt

                def ensure_piece(scp, pi):
                    while issued[scp] < pi + cfg.LA:
                        issue_piece(scp, issued[scp] + 1)
                    return ring_g[scp][pi % nring], ring_p[scp][pi % nring]

                # 3-stage group pipeline:
                #   issue time: sseg = copy(ps_s)            (ACT/DVE)
                #   +1 group:   ps_c = conv+bias; mt=tanh    (PE, ACT)
                #   +2 groups:  ps_m += mt @ mw  (+ close w) (PE)
                def finishA(stage):
                    sseg, ps_s, e, w, deg_t, h_w, ps_m = stage
                    ps_c = psum_cv_pool.tile([128, 128], f32, tag="ps_c",
                                             name="ps_c")
                    if ps_s is not None:
                        nc.tensor.matmul(
                            ps_c[:], convw_t[:, e * 128:(e + 1) * 128],
                            sseg[:], start=True, stop=False)
                    nc.tensor.matmul(
                        ps_c[:], convb_t[:, e * 128:(e + 1) * 128],
                        deg_t[:, e * 128:(e + 1) * 128],
                        start=(ps_s is None), stop=True)
                    mt = smallp.tile([128, 128], f32, tag="mt", bufs=4,
                                     name="mt")
                    nc.scalar.activation(mt[:], ps_c[:], Tanh)
                    return (mt,) + stage

                def finishB(stageA):
                    mt, sseg, ps_s, e, w, deg_t, h_w, ps_m = stageA
                    nc.tensor.matmul(
                        ps_m[:], mt[:], mw_t[:, e * 128:(e + 1) * 128],
                        start=(e == 0), stop=False)
                    if e == ET - 1:
                        nc.tensor.matmul(ps_m[:], ones_f[:], mb_t[:],
                                         start=False, stop=False)
                        nc.tensor.matmul(ps_m[:], ident_t[:], h_w[:],
                                         start=False, stop=True)
                        hnew = smallp.tile([128, 128], f32, tag="hrow",
                                           name="hnew")
                        nc.scalar.activation(hnew[:], ps_m[:], Tanh)
                        if lv < LV - 1:
                            nc.sync.dma_start(
                                h_nxt[w * 128:(w + 1) * 128, :], hnew[:])
                            hb_n = smallp.tile([128, 128], b16, tag="hbrow",
                                               name="hb_n")
                            nc.vector.tensor_copy(hb_n[:], hnew[:])
                            nc.sync.dma_start(
                                hb_nxt[w * 128:(w + 1) * 128, :], hb_n[:])
                        else:
                            nc.sync.dma_start(
                                out_d[w * 128:(w + 1) * 128, :], hnew[:])

                deg_ts = {}
                h_ts = {}

                def prefetch_w(w):
                    if w >= W:
                        return
                    deg_ts[w] = smallp.tile([1, ET * 128], f32, tag="deg",
                                            bufs=4, name="deg_t")
                    nc.sync.dma_start(deg_ts[w][:], deg_d[w:w + 1, :])
                    h_ts[w] = smallp.tile([128, 128], f32, tag="hres",
                                          bufs=4, name="h_res")
                    nc.sync.dma_start(h_ts[w][:],
                                      h_cur[w * 128:(w + 1) * 128, :])

                for w in range(min(2, W)):
                    prefetch_w(w)

                stage1 = [None]  # group awaiting finishA (copy in flight)
                stage2 = [None]  # group awaiting finishB (tanh in flight)
                copy_par = [0]

                def push_group(st):
                    # emit finishB(g-2) then finishA(g-1), then hold g
                    if stage2[0] is not None:
                        finishB(stage2[0])
                        stage2[0] = None
                    if stage1[0] is not None:
                        stage2[0] = finishA(stage1[0])
                        stage1[0] = None
                    stage1[0] = st

                def drain_groups():
                    if stage2[0] is not None:
                        finishB(stage2[0])
                        stage2[0] = None
                    if stage1[0] is not None:
                        stage2[0] = finishA(stage1[0])
                        stage1[0] = None
                        finishB(stage2[0])
                        stage2[0] = None

                for w in range(W):
                    prefetch_w(w + 2)
                    deg_t = deg_ts[w]
                    h_w = h_ts[w]
                    ps_m = psum_m_pool.tile([128, 128], f32, tag="ps_m",
                                            name="ps_m")
                    for e in range(ET):
                        nchunks = int(K[e, :, w].sum())
                        if nchunks == 0:
                            ps_s = None
                        else:
                            ps_s = psum_s_pool.tile([128, 128], f32,
                                                    tag="ps_s", name="ps_s")
                        ci_count = 0
                        for scp in range(NREG):
                            kk = int(K[e, scp, w])
                            for k in range(kk):
                                gc = int(group_slot_start[scp, w, e]) \
                                    // 128 + k
                                pi = gc // cfg.PIECE_CH
                                ci = gc % cfg.PIECE_CH
                                gt, pt = ensure_piece(scp, pi)
                                if ("scatter" in ablate and ci_count != 0):
                                    ci_count += 1
                                    continue
                                nc.tensor.matmul(
                                    ps_s[:],
                                    gt[:, ci, :],
                                    pt[:, ci * 128:(ci + 1) * 128],
                                    start=(ci_count == 0),
                                    stop=("scatter" in ablate
                                          or ci_count == nchunks - 1),
                                )
                                ci_count += 1
                        # PSUM -> SBUF copy right at group close; alternate
                        # ACT / DVE to split the copy load
                        sseg = None
                        if ps_s is not None:
                            sseg = smallp.tile([128, 128], f32, tag="sseg",
                                               bufs=4, name="sseg")
                            if copy_par[0] & 1:
                                nc.scalar.activation(sseg[:], ps_s[:], Copy)
                            else:
                                nc.vector.tensor_copy(sseg[:], ps_s[:])
                            copy_par[0] += 1
                        push_group((sseg, ps_s, e, w, deg_t, h_w, ps_m))
                    deg_ts.pop(w - 1, None)
                    h_ts.pop(w - 1, None)
                drain_groups()
                deg_ts.clear()
                h_ts.clear()

                if lv < LV - 1:
                    h_cur = h_nxt
                    hb_cur = hb_nxt

    nc.compile()
    return nc


# ----------------------------------------------------------------------------
# weight packing (shared across cores)
# ----------------------------------------------------------------------------


def pack_weights(cfg, w_n2l, b_n2l, conv_w, conv_b, merge_w, merge_b):
    LV, ET = cfg.LV, cfg.ET
    packed = {
        "w_n2l": np.asarray(w_n2l, np.float32),
        "b_n2l": np.asarray(b_n2l, np.float32).reshape(1, 128),
        "ident": np.eye(128, dtype=np.float32),
        "conv_w": np.asarray(conv_w, np.float32),
        "conv_b": np.asarray(conv_b, np.float32).reshape(LV, 1, ET * 128),
        "merge_w": np.ascontiguousarray(
            np.asarray(merge_w, np.float32)
            .reshape(LV, ET, 128, 128).transpose(0, 2, 1, 3)
            .reshape(LV, 128, ET * 128)),
        "merge_b": np.asarray(merge_b, np.float32).reshape(LV, 1, 128),
    }
    return packed


def make_in_maps(cfg, node_feat, idx_arrs, off_arrs, deg_arrs, packed):
    in_maps = []
    nf = np.asarray(node_feat, np.float32)
    for c in range(cfg.NC):
        shard = np.zeros((cfg.SP, 128), np.float32)
        shard[: cfg.S] = nf[c * cfg.S:(c + 1) * cfg.S]
        m = {
            "node_feat_t": np.ascontiguousarray(shard.T),
            "idx": idx_arrs[c],
            "dstoff": off_arrs[c],
            "deg": deg_arrs[c],
        }
        m.update(packed)
        in_maps.append(m)
    return in_maps


# ----------------------------------------------------------------------------
# entry point
# ----------------------------------------------------------------------------


def kernel(node_feat, edge_src, edge_dst, w_n2l, b_n2l, conv_w, conv_b,
           merge_w, merge_b):
    from concourse.bass_utils import run_bass_kernel_spmd

    cfg = Cfg()
    plan, idx_arrs, off_arrs, deg_arrs = preprocess_edges(
        cfg, edge_src, edge_dst)
    packed = pack_weights(cfg, w_n2l, b_n2l, conv_w, conv_b, merge_w,
                          merge_b)
    in_maps = make_in_maps(cfg, node_feat, idx_arrs, off_arrs, deg_arrs,
                           packed)
    nc = build_program(cfg, plan)
    res = run_bass_kernel_spmd(nc, in_maps, core_ids=list(range(cfg.NC)))
    out = np.empty((cfg.N, 128), np.float32)
    for c in range(cfg.NC):
        out[c * cfg.S:(c + 1) * cfg.S] = res.results[c]["out"][: cfg.S]
    return out


# revision 73
# speedup vs baseline: 1.0248x; 1.0248x over previous
"""GNN message-passing kernel for 8 Trainium2 NeuronCores (v4).

Reference semantics:
    h = tanh(node_feat @ w_n2l + b_n2l)
    for lv in range(3):
        conv = (h @ conv_w[lv] + conv_b[lv]).reshape(N, 4, D)
        msgs[e] = segment_sum(conv[:, e, :][src_e], dst_e, N)
        msg = tanh(concat_e(msgs))
        h = tanh(msg @ merge_w[lv] + merge_b[lv] + h)

v4 design (h-gather, region streams, PSUM-resident merge):
  - Each level: AllGather the raw h shard in bf16 -> hfull [8*SP, 128]
    (25.7MB).  Conv is applied AFTER the per-type segment sum
    (linearity): msgs_e^T = conv_w_e^T @ seg_e^T + conv_b_e x deg_e.
  - Gather elements are whole h rows (256B), so one gather stream per
    src pair-region serves ALL 4 edge types; slots are ordered by
    (dst block w, edge type e).  4 streams ride 4 SWDGE queues.
  - Loop order w-outer / e-inner: the merge matmul accumulates all 4
    types (+ merge bias + residual via identity matmul) in one PSUM
    tile per block - no SBUF accumulator, no separate merge pass.
  - Finish chains are deferred one group so ACT/PE ping-pong overlaps
    the next group's scatter matmuls.
  - Pad slots use idx=0 / dstoff=-1 (zero one-hot column).

Distribution: nodes sharded over 8 cores (12500 each). Weights replicated.
"""

import math
import numpy as np
import ml_dtypes

BF16 = ml_dtypes.bfloat16

# ----------------------------------------------------------------------------
# configuration
# ----------------------------------------------------------------------------


class Cfg:
    def __init__(self, n_nodes=100000, n_cores=8, ept=1600000, piece_chunks=16,
                 nqueues=4, sort_src=True, single_packet=False, lookahead=2,
                 rotate_q=False):
        self.N = n_nodes
        self.NC = n_cores
        self.ET = 4          # edge types
        self.LV = 3          # levels
        self.D = 128         # feature dim (latent == feats == 128)
        self.EPT = ept       # edges per type
        self.S = self.N // self.NC            # real nodes per core
        self.SP = ((self.S + 127) // 128) * 128   # padded nodes per core
        self.W = self.SP // 128               # dst blocks per core
        self.NREG = self.NC // 2              # src core-pair regions
        self.REG_ROWS = 2 * self.SP           # rows per pair region
        assert self.REG_ROWS - 1 <= 32767, "int16 gather index limit"
        self.PIECE_CH = piece_chunks          # chunks per gather piece
        self.PIECE = self.PIECE_CH * 128      # slots per gather piece
        self.NQ = nqueues                     # SWDGE queues for gathers
        self.SORT_SRC = sort_src              # sort slots by src within group
        self.SINGLE_PACKET = single_packet
        self.LA = lookahead                   # gather piece lookahead depth
        self.ROT_Q = rotate_q                 # rotate queues across pieces


# ----------------------------------------------------------------------------
# CPU preprocessing: edge bucketing, capacities, index/offset arrays
# ----------------------------------------------------------------------------


def preprocess_edges(cfg, edge_src, edge_dst):
    """Bucket edges by (dst core, src pair-region, dst block, edge type).

    One stream per src pair-region; slots ordered by (w, e, rank).
    Returns a plan shared by all cores (capacities, piece table) plus
    per-core idx / dstoff arrays.
    """
    NC, ET, W, S, SP = cfg.NC, cfg.ET, cfg.W, cfg.S, cfg.SP
    NREG = cfg.NREG

    m = np.zeros((NC, ET, NREG, W), dtype=np.int64)
    per_core_edges = [[None] * ET for _ in range(NC)]  # (scp, w, idx16, doff)
    for e in range(ET):
        src = np.asarray(edge_src[e], dtype=np.int64)
        dst = np.asarray(edge_dst[e], dtype=np.int64)
        owner = dst // S
        dl = dst - owner * S
        w = dl >> 7
        scp = src // (2 * S)
        src_in_pair = src - scp * (2 * S)
        idx16 = (src_in_pair // S) * SP + (src_in_pair % S)
        doff = dl & 127
        key = (owner * NREG + scp) * W + w
        cnt = np.bincount(key, minlength=NC * NREG * W).reshape(NC, NREG, W)
        m[:, e] = cnt
        order = np.argsort(key, kind="stable")
        ksorted = key[order]
        bounds = np.searchsorted(ksorted, np.arange(NC) * NREG * W)
        bounds = np.append(bounds, len(ksorted))
        for c in range(NC):
            sl = order[bounds[c]:bounds[c + 1]]
            per_core_edges[c][e] = (
                scp[sl], w[sl], idx16[sl].astype(np.int16),
                doff[sl].astype(np.int16)
            )

    # capacities (chunks of 128) per (e, scp, w): max over cores
    K = np.ceil(m.max(axis=0) / 128.0).astype(np.int64)  # [ET, NREG, W]

    # stream layout per scp: groups ordered by (w, e)
    group_slot_start = np.zeros((NREG, W, ET), dtype=np.int64)
    stream_len = np.zeros(NREG, dtype=np.int64)
    for scp in range(NREG):
        off = 0
        for w in range(W):
            for e in range(ET):
                group_slot_start[scp, w, e] = off
                off += K[e, scp, w] * 128
        stream_len[scp] = off

    # piece table: per stream, pieces of cfg.PIECE slots (last short)
    pieces = []  # (scp, slot_start, n_slots)
    piece_of_stream = {}
    for scp in range(NREG):
        start_list = []
        off = 0
        while off < stream_len[scp]:
            n = min(cfg.PIECE, stream_len[scp] - off)
            n = ((n + 127) // 128) * 128
            start_list.append((len(pieces), off, n))
            pieces.append((scp, off, n))
            off += n
        piece_of_stream[scp] = start_list

    npieces = len(pieces)
    idx_cols = cfg.PIECE // 16
    off_cols = cfg.PIECE_CH

    # per-core arrays
    idx_arrs = []
    off_arrs = []
    deg_arrs = []
    for c in range(NC):
        idx_flat = {scp: np.zeros(stream_len[scp], dtype=np.int16)
                    for scp in range(NREG)}
        off_flat = {scp: np.full(stream_len[scp], -1, dtype=np.int16)
                    for scp in range(NREG)}
        deg = np.zeros((cfg.ET, cfg.SP), dtype=np.float32)
        for e in range(ET):
            escp, ew, eidx, edoff = per_core_edges[c][e]
            gkey = escp * W + ew
            if cfg.SORT_SRC:
                order = np.lexsort((eidx, gkey))
            else:
                order = np.argsort(gkey, kind="stable")
            gs = gkey[order]
            grp_start_pos = np.searchsorted(gs, gs, side="left")
            rank = np.arange(len(gs)) - grp_start_pos
            slot = group_slot_start[escp[order], ew[order], e] + rank
            for scp in range(NREG):
                msk = escp[order] == scp
                idx_flat[scp][slot[msk]] = eidx[order][msk]
                off_flat[scp][slot[msk]] = edoff[order][msk]
            dln = ew * 128 + edoff
            deg[e] = np.bincount(dln, minlength=cfg.SP).astype(np.float32)
        # wrap into DMA layouts
        idx_arr = np.zeros((npieces, 128, idx_cols), dtype=np.int16)
        off_arr = np.full((npieces, 128, off_cols), -1, dtype=np.int16)
        for scp, plist in piece_of_stream.items():
            fl_i = idx_flat[scp]
            fl_o = off_flat[scp]
            for (pid, off0, n) in plist:
                chunk = np.zeros(n, dtype=np.int16)
                avail = min(n, len(fl_i) - off0)
                chunk[:avail] = fl_i[off0:off0 + avail]
                wrapped = chunk.reshape(-1, 16).T          # [16, n/16]
                idx_arr[pid, :, : n // 16] = np.tile(wrapped, (8, 1))
                oc = np.full(n, -1, dtype=np.int16)
                oc[:avail] = fl_o[off0:off0 + avail]
                oc = oc.reshape(-1, 128).T                # [128, n/128]
                off_arr[pid, :, : n // 128] = oc
        idx_arrs.append(idx_arr)
        off_arrs.append(off_arr)
        # deg packed per w: [W, ET*128]
        degw = np.ascontiguousarray(
            deg.reshape(ET, W, 128).transpose(1, 0, 2).reshape(W, ET * 128))
        deg_arrs.append(degw)

    plan = {
        "K": K,
        "group_slot_start": group_slot_start,
        "stream_len": stream_len,
        "pieces": pieces,
        "piece_of_stream": piece_of_stream,
        "npieces": npieces,
        "idx_cols": idx_cols,
        "off_cols": off_cols,
    }
    return plan, idx_arrs, off_arrs, deg_arrs


# ----------------------------------------------------------------------------
# program builder
# ----------------------------------------------------------------------------


def build_program(cfg, plan, ablate=()):
    ablate = set(ablate)
    from concourse import bass, bacc, tile, mybir

    f32 = mybir.dt.float32
    b16 = mybir.dt.bfloat16
    i16 = mybir.dt.int16
    Tanh = mybir.ActivationFunctionType.Tanh
    Copy = mybir.ActivationFunctionType.Copy

    NC, ET, W, LV = cfg.NC, cfg.ET, cfg.W, cfg.LV
    NREG = cfg.NREG
    SP = cfg.SP
    K = plan["K"]
    group_slot_start = plan["group_slot_start"]
    piece_of_stream = plan["piece_of_stream"]
    npieces = plan["npieces"]
    idx_cols = plan["idx_cols"]
    off_cols = plan["off_cols"]

    nc = bacc.Bacc("TRN2", target_bir_lowering=False, debug=False,
                   num_devices=NC, num_swdge_queues=cfg.NQ)

    # ---- external tensors ----
    nf_t = nc.dram_tensor("node_feat_t", [128, SP], f32, kind="ExternalInput")
    idx_d = nc.dram_tensor("idx", [npieces, 128, idx_cols], i16,
                           kind="ExternalInput")
    off_d = nc.dram_tensor("dstoff", [npieces, 128, off_cols], i16,
                           kind="ExternalInput")
    deg_d = nc.dram_tensor("deg", [W, ET * 128], f32, kind="ExternalInput")
    w_n2l_d = nc.dram_tensor("w_n2l", [128, 128], f32, kind="ExternalInput")
    b_n2l_d = nc.dram_tensor("b_n2l", [1, 128], f32, kind="ExternalInput")
    ident_d = nc.dram_tensor("ident", [128, 128], f32, kind="ExternalInput")
    convw_d = nc.dram_tensor("conv_w", [LV, 128, ET * 128], f32,
                             kind="ExternalInput")
    convb_d = nc.dram_tensor("conv_b", [LV, 1, ET * 128], f32,
                             kind="ExternalInput")
    mw_d = nc.dram_tensor("merge_w", [LV, 128, ET * 128], f32,
                          kind="ExternalInput")
    mb_d = nc.dram_tensor("merge_b", [LV, 1, 128], f32, kind="ExternalInput")
    out_d = nc.dram_tensor("out", [SP, 128], f32, kind="ExternalOutput")

    with tile.TileContext(nc) as tc:
        with (
            tc.tile_pool(name="dram2", bufs=2, space="DRAM") as dramp2,
            tc.tile_pool(name="const", bufs=1) as constp,
            tc.tile_pool(name="wts", bufs=2) as wtsp,
            tc.tile_pool(name="gather", bufs=cfg.LA + 1) as gatherp,
            tc.tile_pool(name="ponehot", bufs=cfg.LA + 1) as ponep,
            tc.tile_pool(name="idxp", bufs=cfg.LA + 1) as idxp,
            tc.tile_pool(name="small", bufs=3) as smallp,
            tc.tile_pool(name="psum_s", bufs=3, space="PSUM") as psum_s_pool,
            tc.tile_pool(name="psum_m", bufs=2, space="PSUM") as psum_m_pool,
            tc.tile_pool(name="psum_cv", bufs=3, space="PSUM") as psum_cv_pool,
        ):
            # ---- DRAM intermediates ----
            hb_cur = dramp2.tile([SP, 128], b16, tag="hb", name="hb_cur")
            h_cur = dramp2.tile([SP, 128], f32, tag="h_cur", name="h_cur")

            # ---- constants ----
            iota_t = constp.tile([128, 128], i16)
            nc.gpsimd.iota(iota_t[:], pattern=[[1, 128]], base=0,
                           channel_multiplier=0)
            ones_f = constp.tile([1, 128], f32)
            nc.vector.memset(ones_f[:], 1.0)
            ident_t = constp.tile([128, 128], f32)
            nc.sync.dma_start(ident_t[:], ident_d[:])
            w_n2l_t = constp.tile([128, 128], f32)
            nc.sync.dma_start(w_n2l_t[:], w_n2l_d[:])
            b_n2l_t = constp.tile([1, 128], f32)
            nc.sync.dma_start(b_n2l_t[:], b_n2l_d[:])

            # ---- embed: h0 = tanh(nf @ w_n2l + b_n2l) ----
            # nf tile is the stationary operand, so the result lands in
            # row layout [node, f] directly - no transpose pass.
            for w in range(W):
                nf_w = smallp.tile([128, 128], f32, tag="nf", name="nf_w")
                nc.sync.dma_start(nf_w[:], nf_t[:, w * 128:(w + 1) * 128])
                ps_h = psum_m_pool.tile([128, 128], f32, tag="ps_m",
                                        name="ps_h")
                nc.tensor.matmul(ps_h[:], nf_w[:], w_n2l_t[:],
                                 start=True, stop=False)
                nc.tensor.matmul(ps_h[:], ones_f[:], b_n2l_t[:],
                                 start=False, stop=True)
                h_w = smallp.tile([128, 128], f32, tag="hrow", name="h_w")
                nc.scalar.activation(h_w[:], ps_h[:], Tanh)
                nc.sync.dma_start(h_cur[w * 128:(w + 1) * 128, :], h_w[:])
                hb_w = smallp.tile([128, 128], b16, tag="hbrow", name="hb_w")
                nc.scalar.activation(hb_w[:], ps_h[:], Tanh)
                nc.sync.dma_start(hb_cur[w * 128:(w + 1) * 128, :], hb_w[:])

            # ---- level loop ----
            for lv in range(LV if "levels" not in ablate else 0):
                hfull = dramp2.tile([NC * SP, 128], b16, tag="hfull",
                                    name="hfull", addr_space="Shared")
                if "allgather" not in ablate:
                    nc.gpsimd.collective_compute(
                        "AllGather",
                        bass.mybir.AluOpType.bypass,
                        replica_groups=[list(range(NC))],
                        ins=[hb_cur[:].opt()],
                        outs=[hfull[:].opt()],
                    )

                convw_t = wtsp.tile([128, ET * 128], f32, tag="convw",
                                    name="convw_t")
                nc.sync.dma_start(convw_t[:], convw_d[lv])
                convb_t = wtsp.tile([1, ET * 128], f32, tag="convb",
                                    name="convb_t")
                nc.sync.dma_start(convb_t[:], convb_d[lv])
                mw_t = wtsp.tile([128, ET * 128], f32, tag="mw", name="mw_t")
                nc.sync.dma_start(mw_t[:], mw_d[lv])
                mb_t = wtsp.tile([1, 128], f32, tag="mb", name="mb_t")
                nc.sync.dma_start(mb_t[:], mb_d[lv])

                h_nxt = (dramp2.tile([SP, 128], f32, tag="h_cur",
                                     name="h_nxt")
                         if lv < LV - 1 else None)
                hb_nxt = (dramp2.tile([SP, 128], b16, tag="hb",
                                      name="hb_nxt")
                          if lv < LV - 1 else None)

                # ---- per-level gather stream state ----
                nring = cfg.LA + 1
                issued = [-1] * NREG
                ring_g = [[None] * nring for _ in range(NREG)]
                ring_p = [[None] * nring for _ in range(NREG)]

                def issue_piece(scp, pi):
                    plist = piece_of_stream[scp]
                    if pi >= len(plist):
                        issued[scp] = pi
                        return
                    gpid, off0, n = plist[pi]
                    nch = n // 128
                    it = idxp.tile([128, idx_cols], i16, tag=f"idx{scp}",
                                   name="it")
                    if "gather" not in ablate:
                        nc.sync.dma_start(it[:, : n // 16],
                                          idx_d[gpid][:, : n // 16])
                    ot = idxp.tile([128, off_cols], i16, tag=f"off{scp}",
                                   name="ot")
                    if "onehot" not in ablate:
                        nc.sync.dma_start(ot[:, :nch], off_d[gpid][:, :nch])
                    gt = gatherp.tile([128, cfg.PIECE_CH, 128], b16,
                                      tag=f"g{scp}", name="gt")
                    reg_base = scp * cfg.REG_ROWS
                    if "gather" in ablate:
                        nc.vector.memset(gt[:, :nch, 0:1], 0.0)
                    else:
                        nc.gpsimd.dma_gather(
                            out_ap=gt[:, :nch, :],
                            in_ap=hfull[reg_base:reg_base + cfg.REG_ROWS, :],
                            idxs_ap=it[:, : n // 16],
                            num_idxs=n,
                            num_idxs_reg=n,
                            elem_size=128,
                            elem_step=128,
                            single_packet=cfg.SINGLE_PACKET,
                            queue_num=((scp + pi) % cfg.NQ if cfg.ROT_Q
                                       else scp % cfg.NQ),
                        )
                    pt = ponep.tile([128, cfg.PIECE_CH * 128], b16,
                                    tag=f"p{scp}", name="pt")
                    if "onehot" in ablate:
                        nc.vector.memset(pt[:, 0:1], 0.0)
                    else:
                        nc.vector.tensor_tensor(
                            out=pt[:, : nch * 128].rearrange(
                                "p (c e) -> p c e", e=128),
                            in0=ot[:, :nch].unsqueeze(2).to_broadcast(
                                [128, nch, 128]),
                            in1=iota_t[:].unsqueeze(1).to_broadcast(
                                [128, nch, 128]),
                            op=bass.mybir.AluOpType.is_equal,
                        )
                    issued[scp] = pi
                    ring_g[scp][pi % nring] = gt
                    ring_p[scp][pi % nring] = pt

                def ensure_piece(scp, pi):
                    while issued[scp] < pi + cfg.LA:
                        issue_piece(scp, issued[scp] + 1)
                    return ring_g[scp][pi % nring], ring_p[scp][pi % nring]

                # 3-stage group pipeline:
                #   issue time: sseg = copy(ps_s)            (ACT/DVE)
                #   +1 group:   ps_c = conv+bias; mt=tanh    (PE, ACT)
                #   +2 groups:  ps_m += mt @ mw  (+ close w) (PE)
                def finishA(stage):
                    sseg, ps_s, e, w, deg_t, h_w, ps_m = stage
                    ps_c = psum_cv_pool.tile([128, 128], f32, tag="ps_c",
                                             name="ps_c")
                    if ps_s is not None:
                        nc.tensor.matmul(
                            ps_c[:], convw_t[:, e * 128:(e + 1) * 128],
                            sseg[:], start=True, stop=False)
                    nc.tensor.matmul(
                        ps_c[:], convb_t[:, e * 128:(e + 1) * 128],
                        deg_t[:, e * 128:(e + 1) * 128],
                        start=(ps_s is None), stop=True)
                    mt = smallp.tile([128, 128], f32, tag="mt", bufs=4,
                                     name="mt")
                    nc.scalar.activation(mt[:], ps_c[:], Tanh)
                    return (mt,) + stage

                def finishB(stageA):
                    mt, sseg, ps_s, e, w, deg_t, h_w, ps_m = stageA
                    nc.tensor.matmul(
                        ps_m[:], mt[:], mw_t[:, e * 128:(e + 1) * 128],
                        start=(e == 0), stop=False)
                    if e == ET - 1:
                        nc.tensor.matmul(ps_m[:], ones_f[:], mb_t[:],
                                         start=False, stop=False)
                        nc.tensor.matmul(ps_m[:], ident_t[:], h_w[:],
                                         start=False, stop=True)
                        hnew = smallp.tile([128, 128], f32, tag="hrow",
                                           name="hnew")
                        nc.scalar.activation(hnew[:], ps_m[:], Tanh)
                        if lv < LV - 1:
                            nc.sync.dma_start(
                                h_nxt[w * 128:(w + 1) * 128, :], hnew[:])
                            hb_n = smallp.tile([128, 128], b16, tag="hbrow",
                                               name="hb_n")
                            nc.vector.tensor_copy(hb_n[:], hnew[:])
                            nc.sync.dma_start(
                                hb_nxt[w * 128:(w + 1) * 128, :], hb_n[:])
                        else:
                            nc.sync.dma_start(
                                out_d[w * 128:(w + 1) * 128, :], hnew[:])

                deg_ts = {}
                h_ts = {}

                def prefetch_w(w):
                    if w >= W:
                        return
                    deg_ts[w] = smallp.tile([1, ET * 128], f32, tag="deg",
                                            bufs=4, name="deg_t")
                    nc.sync.dma_start(deg_ts[w][:], deg_d[w:w + 1, :])
                    h_ts[w] = smallp.tile([128, 128], f32, tag="hres",
                                          bufs=4, name="h_res")
                    nc.sync.dma_start(h_ts[w][:],
                                      h_cur[w * 128:(w + 1) * 128, :])

                for w in range(min(2, W)):
                    prefetch_w(w)

                stage1 = [None]  # group awaiting finishA (copy in flight)
                stage2 = [None]  # group awaiting finishB (tanh in flight)
                copy_par = [0]

                def push_group(st):
                    # emit finishB(g-2) then finishA(g-1), then hold g
                    if stage2[0] is not None:
                        finishB(stage2[0])
                        stage2[0] = None
                    if stage1[0] is not None:
                        stage2[0] = finishA(stage1[0])
                        stage1[0] = None
                    stage1[0] = st

                def drain_groups():
                    if stage2[0] is not None:
                        finishB(stage2[0])
                        stage2[0] = None
                    if stage1[0] is not None:
                        stage2[0] = finishA(stage1[0])
                        stage1[0] = None
                        finishB(stage2[0])
                        stage2[0] = None

                for w in range(W):
                    prefetch_w(w + 2)
                    deg_t = deg_ts[w]
                    h_w = h_ts[w]
                    ps_m = psum_m_pool.tile([128, 128], f32, tag="ps_m",
                                            name="ps_m")
                    for e in range(ET):
                        nchunks = int(K[e, :, w].sum())
                        if nchunks == 0:
                            ps_s = None
                        else:
                            ps_s = psum_s_pool.tile([128, 128], f32,
                                                    tag="ps_s", name="ps_s")
                        ci_count = 0
                        for scp in range(NREG):
                            kk = int(K[e, scp, w])
                            for k in range(kk):
                                gc = int(group_slot_start[scp, w, e]) \
                                    // 128 + k
                                pi = gc // cfg.PIECE_CH
                                ci = gc % cfg.PIECE_CH
                                gt, pt = ensure_piece(scp, pi)
                                if ("scatter" in ablate and ci_count != 0):
                                    ci_count += 1
                                    continue
                                nc.tensor.matmul(
                                    ps_s[:],
                                    gt[:, ci, :],
                                    pt[:, ci * 128:(ci + 1) * 128],
                                    start=(ci_count == 0),
                                    stop=("scatter" in ablate
                                          or ci_count == nchunks - 1),
                                )
                                ci_count += 1
                        # PSUM -> SBUF copy right at group close; alternate
                        # ACT / DVE to split the copy load
                        sseg = None
                        if ps_s is not None:
                            sseg = smallp.tile([128, 128], f32, tag="sseg",
                                               bufs=4, name="sseg")
                            if copy_par[0] & 1:
                                nc.scalar.activation(sseg[:], ps_s[:], Copy)
                            else:
                                nc.vector.tensor_copy(sseg[:], ps_s[:])
                            copy_par[0] += 1
                        push_group((sseg, ps_s, e, w, deg_t, h_w, ps_m))
                    deg_ts.pop(w - 1, None)
                    h_ts.pop(w - 1, None)
                drain_groups()
                deg_ts.clear()
                h_ts.clear()

                if lv < LV - 1:
                    h_cur = h_nxt
                    hb_cur = hb_nxt

    nc.compile()
    return nc


# ----------------------------------------------------------------------------
# weight packing (shared across cores)
# ----------------------------------------------------------------------------


def pack_weights(cfg, w_n2l, b_n2l, conv_w, conv_b, merge_w, merge_b):
    LV, ET = cfg.LV, cfg.ET
    packed = {
        "w_n2l": np.asarray(w_n2l, np.float32),
        "b_n2l": np.asarray(b_n2l, np.float32).reshape(1, 128),
        "ident": np.eye(128, dtype=np.float32),
        "conv_w": np.asarray(conv_w, np.float32),
        "conv_b": np.asarray(conv_b, np.float32).reshape(LV, 1, ET * 128),
        "merge_w": np.ascontiguousarray(
            np.asarray(merge_w, np.float32)
            .reshape(LV, ET, 128, 128).transpose(0, 2, 1, 3)
            .reshape(LV, 128, ET * 128)),
        "merge_b": np.asarray(merge_b, np.float32).reshape(LV, 1, 128),
    }
    return packed


def make_in_maps(cfg, node_feat, idx_arrs, off_arrs, deg_arrs, packed):
    in_maps = []
    nf = np.asarray(node_feat, np.float32)
    for c in range(cfg.NC):
        shard = np.zeros((cfg.SP, 128), np.float32)
        shard[: cfg.S] = nf[c * cfg.S:(c + 1) * cfg.S]
        m = {
            "node_feat_t": np.ascontiguousarray(shard.T),
            "idx": idx_arrs[c],
            "dstoff": off_arrs[c],
            "deg": deg_arrs[c],
        }
        m.update(packed)
        in_maps.append(m)
    return in_maps


# ----------------------------------------------------------------------------
# entry point
# ----------------------------------------------------------------------------


def kernel(node_feat, edge_src, edge_dst, w_n2l, b_n2l, conv_w, conv_b,
           merge_w, merge_b):
    from concourse.bass_utils import run_bass_kernel_spmd

    cfg = Cfg()
    plan, idx_arrs, off_arrs, deg_arrs = preprocess_edges(
        cfg, edge_src, edge_dst)
    packed = pack_weights(cfg, w_n2l, b_n2l, conv_w, conv_b, merge_w,
                          merge_b)
    in_maps = make_in_maps(cfg, node_feat, idx_arrs, off_arrs, deg_arrs,
                           packed)
    nc = build_program(cfg, plan)
    res = run_bass_kernel_spmd(nc, in_maps, core_ids=list(range(cfg.NC)))
    out = np.empty((cfg.N, 128), np.float32)
    for c in range(cfg.NC):
        out[c * cfg.S:(c + 1) * cfg.S] = res.results[c]["out"][: cfg.S]
    return out
